# revision 2
# baseline (speedup 1.0000x reference)
"""Bahdanau additive attention on 8 TRN2 NeuronCores — v6.

Reference computation (B=32, S=2048, H=512):
    q_proj = query @ Wa_w.T + Wa_b                  # [B,1,H]
    k_proj = keys @ Ua_w.T + Ua_b                   # [B,S,H]
    scores = tanh(q_proj + k_proj) @ Va_w.T + Va_b  # [B,S,1]
    weights = softmax(scores, axis=1)
    out = weights.T @ keys                          # [B,1,H]

Sharding: data-parallel over batch, 4 batches per core, params replicated.

Platform model (microbenched on the axon-tunneled cores): execution is
dispatch-bound and effectively serialized across engines. A matmul costs
~55us regardless of width; LDWEIGHTS ~3us; 2D ACT/DVE ops ~30-80us almost
independent of width; engine-queue switches add ~50-100us; DMA latency
~165us but amortized when few and large. So the kernel minimizes the
instruction count and engine switches:

  - 256 matmuls/core is the bf16 floor (512-col moving cap, 128-row
    contraction, 4x4 h-chunking, 8192 moving columns). fp8 DoubleRow would
    halve it but measures 2.5e-2 rel err vs the 2e-2 gate.
  - Redundant LDWEIGHTS are deleted post-compile (PE array retains its
    stationary operand): 256 -> 64.
  - keys live in SBUF for the whole kernel via 2 big contiguous DMAs.
  - tanh: wide [128,2048] 2D activations, qb bias fused per-partition;
    emitted in back-to-back pairs so PE<->ACT switches halve.
  - scores: OFF the PE, batched across batch PAIRS: 4 scalar_tensor_tensor
    (3D, vat per-partition scalars) + 1 GpSimd partition_all_reduce
    [128,4096] (sum over partitions, replicated result) + 1 wide exp.
    Va_b is dropped: constant score shifts cancel in softmax.
  - weighted sum per pair: 1 DVE mul (4D, w broadcast) + 1 grouped reduce
    + 1 denominator reduce over the same keys tile. No second keys load.
  - output: ONE [128, 4, 5] f32 DMA; host divides by the denominators.
  - stage-shifting keeps each in-order queue from parking on cross-engine
    chains (scores/output of pair p emit during pair p+1's matmuls).

~380 BIR instructions/core vs 1068 in the previous kernel.
"""

import numpy as np
import ml_dtypes

import concourse.bass as bass
import concourse.tile as tile
from concourse import bacc, mybir, bass_isa
from concourse.bass_utils import run_bass_kernel_spmd

B, S, H = 32, 2048, 512
NCORES = 8
BPC = B // NCORES  # batches per core
P = 128
HC = H // P        # 4 h-chunks
SC = S // 512      # 4 column chunks per batch
NPAIR = BPC // 2

F32 = mybir.dt.float32
BF16 = mybir.dt.bfloat16
BF16_NP = ml_dtypes.bfloat16

_CACHED_NC = {}


def _dedup_ldweights(nc):
    """Remove back-to-back InstLdweights with identical weight APs.

    The PE array keeps its stationary operand until the next LDWEIGHTS, so
    consecutive matmuls sharing a (constant) weights tile only need the
    first load. Safe here: the weights tile is written once, there are no
    transpose matmuls, and the dropped instructions carry no sync_info
    (semaphore waits/updates all live on the matmuls).
    """
    removed = 0
    for f in nc.m.functions:
        for blk in f.blocks:
            insts = blk.instructions
            keep = []
            last_sig = None
            for inst in insts:
                nm = type(inst).__name__
                if nm == "InstLdweights":
                    sig = (
                        str(inst.ins),
                        str(inst.perf_mode),
                        str(inst.is_transpose),
                        str(inst.tile_position),
                        str(inst.tile_size),
                    )
                    if sig == last_sig and inst.sync_info is None:
                        removed += 1
                        continue
                    last_sig = sig
                elif nm == "InstMatmult":
                    if inst.is_transpose:
                        last_sig = None
                elif nm in ("InstCall", "InstUnconditionalBranch",
                            "InstCompareAndBranch", "InstISA"):
                    # control flow: stop tracking across block transitions.
                    # Other engines' instructions can't clobber PE weights.
                    last_sig = None
                keep.append(inst)
            if removed:
                insts[:] = keep
    return removed


def build_nc(repeat=1):
    nc = bacc.Bacc()

    # keysT packed [p, b, hc, s]: one contiguous region per batch.
    keyst_ext = nc.declare_dram_parameter("keysT16", [P, BPC, HC, S], BF16,
                                          isOutput=False)
    # bf16 consts: uaT [p, hc*H]
    consts_ext = nc.declare_dram_parameter("consts16", [P, HC * H], BF16,
                                           isOutput=False)
    # f32 consts: vat [p, hc] + qb [p, hc*b]
    consts32_ext = nc.declare_dram_parameter("consts32", [P, HC + HC * BPC], F32,
                                             isOutput=False)
    # out: [p, b, hc(4) + den(1)] f32
    out_ext = nc.declare_dram_parameter("out_raw", [P, BPC, HC + 1], F32,
                                        isOutput=True)

    mult = mybir.AluOpType.mult
    add = mybir.AluOpType.add

    with tile.TileContext(nc) as tc:
        with (
            tc.tile_pool(name="consts", bufs=1) as cpool,
            tc.tile_pool(name="t_p", bufs=1) as t_p,
            tc.tile_pool(name="sp_p", bufs=1) as sp_p,
            tc.tile_pool(name="sall_p", bufs=1) as sall_p,
            tc.tile_pool(name="w_p", bufs=2) as w_p,
            tc.tile_pool(name="prod_p", bufs=1) as prod_p,
            tc.tile_pool(name="kp_ps", bufs=1, space="PSUM") as kp_ps,
        ):
            # ---- constants + keys (4 DMAs total, outside the hot loop)
            call = cpool.tile([P, HC * H], BF16)
            nc.sync.dma_start(out=call, in_=consts_ext[:, :])
            uat = call.rearrange("p (hc o) -> p hc o", hc=HC)
            call32 = cpool.tile([P, HC + HC * BPC], F32)
            nc.scalar.dma_start(out=call32, in_=consts32_ext[:, :])
            vat = call32[:, 0:HC]                                   # [p, hc]
            qbt = call32[:, HC:].rearrange("p (hc b) -> p hc b", hc=HC)

            ktt = cpool.tile([P, BPC, HC, S], BF16)
            half = BPC // 2
            nc.sync.dma_start(out=ktt[:, 0:half], in_=keyst_ext[:, 0:half])
            nc.scalar.dma_start(out=ktt[:, half:], in_=keyst_ext[:, half:])

            outsb = cpool.tile([P, BPC, HC + 1], F32)

            # stage-shifting: slot k's scheduled work is emitted right
            # before slot k+offset's kproj.
            slots = {}
            flush_idx = 0

            def schedule(offset, fn):
                slots.setdefault(flush_idx + offset, []).append(fn)

            def flush(idx):
                for fn in slots.pop(idx, ()):
                    fn()

            for r in range(repeat):
                for pr in range(NPAIR):
                    b0 = pr * 2
                    flush(flush_idx)

                    # t_pair[p, j, oc, s] for the two batches of this pair.
                    # Both batches' accumulators live in PSUM at once (8
                    # banks) and share each (oc,hc) weights tile across 8
                    # consecutive matmuls: LDWEIGHTS dedups 8:1 and the
                    # same-bank accumulate gap is 8.
                    t_pair = t_p.tile([P, 2, HC, S], BF16, tag="t")
                    for oc in range(HC):
                        kp0 = kp_ps.tile([P, S], F32, tag="kp0")
                        kp1 = kp_ps.tile([P, S], F32, tag="kp1")
                        for hc in range(HC):
                            for j, kp in ((0, kp0), (1, kp1)):
                                for sc in range(SC):
                                    nc.tensor.matmul(
                                        kp[:, sc * 512:(sc + 1) * 512],
                                        uat[:, hc, oc * P:(oc + 1) * P],
                                        ktt[:, b0 + j, hc,
                                            sc * 512:(sc + 1) * 512],
                                        start=(hc == 0),
                                        stop=(hc == HC - 1),
                                    )
                        # two wide tanh back-to-back: one PE->ACT switch
                        # per 32 matmuls
                        for j, kp in ((0, kp0), (1, kp1)):
                            nc.scalar.activation(
                                out=t_pair[:, j, oc, :],
                                in_=kp,
                                func=mybir.ActivationFunctionType.Tanh,
                                bias=qbt[:, oc, b0 + j:b0 + j + 1],
                            )

                    def stage2(b0=b0, t_pair=t_pair):
                        # scores = sum_oc vat_oc * t_oc for both batches
                        sp_prev = None
                        for oc in range(HC):
                            sp = sp_p.tile([P, 2, S], F32,
                                           tag=f"sp{oc % 2}")
                            if oc == 0:
                                nc.vector.tensor_scalar_mul(
                                    sp, t_pair[:, :, 0, :], vat[:, 0:1])
                            else:
                                nc.vector.scalar_tensor_tensor(
                                    out=sp, in0=t_pair[:, :, oc, :],
                                    scalar=vat[:, oc:oc + 1],
                                    in1=sp_prev, op0=mult, op1=add)
                            sp_prev = sp
                        sall = sall_p.tile([P, 2 * S], F32, tag="sall")
                        nc.gpsimd.partition_all_reduce(
                            sall, sp_prev.rearrange("p j s -> p (j s)"),
                            channels=P, reduce_op=bass_isa.ReduceOp.add)
                        w_sb = w_p.tile([P, 2, S], BF16, tag="w")
                        nc.scalar.activation(
                            out=w_sb.rearrange("p j s -> p (j s)"), in_=sall,
                            func=mybir.ActivationFunctionType.Exp)

                        def stage3(b0=b0, w_sb=w_sb):
                            prod = prod_p.tile([P, 2, HC, S], BF16, tag="prod")
                            nc.vector.tensor_mul(
                                out=prod,
                                in0=ktt[:, b0:b0 + 2],
                                in1=w_sb.unsqueeze(2).broadcast_to(
                                    (P, 2, HC, S)),
                            )
                            nc.vector.reduce_sum(
                                out=outsb[:, b0:b0 + 2, 0:HC],
                                in_=prod,
                                axis=mybir.AxisListType.X,
                            )
                            nc.vector.reduce_sum(
                                out=outsb[:, b0:b0 + 2, HC:HC + 1],
                                in_=w_sb,
                                axis=mybir.AxisListType.X,
                            )

                        schedule(1, stage3)

                    schedule(1, stage2)
                    flush_idx += 1

            while slots:
                flush(min(slots))

            nc.sync.dma_start(out=out_ext[:, :, :], in_=outsb)

    nc.compile()
    _dedup_ldweights(nc)
    return nc


def _get_nc():
    if "nc" not in _CACHED_NC:
        _CACHED_NC["nc"] = build_nc()
    return _CACHED_NC["nc"]


def _host_prep(inputs):
    keys = np.asarray(inputs["keys"], dtype=np.float32)
    query = np.asarray(inputs["query"], dtype=np.float32)
    wa_w = np.asarray(inputs["Wa_w"], dtype=np.float32)
    wa_b = np.asarray(inputs["Wa_b"], dtype=np.float32)
    ua_w = np.asarray(inputs["Ua_w"], dtype=np.float32)
    ua_b = np.asarray(inputs["Ua_b"], dtype=np.float32)
    va_w = np.asarray(inputs["Va_w"], dtype=np.float32)

    # keysT16[p, b, hc, s] = keys[b, s, hc*128+p]
    keysT16 = np.ascontiguousarray(
        keys.reshape(B, S, HC, P).transpose(3, 0, 2, 1)
    ).astype(BF16_NP)  # [P, B, HC, S]

    # uaT[p, hc, o] = Ua_w[o, hc*128+p]  (contraction rows on partitions)
    uaT = np.ascontiguousarray(
        ua_w.T.reshape(HC, P, H).transpose(1, 0, 2)
    ).astype(BF16_NP)  # [P, HC, H]
    # vat[p, hc] = Va_w[0, hc*128+p]
    vat = np.ascontiguousarray(va_w[0].reshape(HC, P).T).astype(np.float32)
    qproj = query[:, 0, :] @ wa_w.T + wa_b + ua_b  # [B, H]
    # qb[p, hc, b] = qproj[b, hc*128+p]
    qb = np.ascontiguousarray(
        qproj.reshape(B, HC, P).transpose(2, 1, 0)
    ).astype(np.float32)  # [P, HC, B]

    in_maps = []
    for c in range(NCORES):
        sl = slice(c * BPC, (c + 1) * BPC)
        consts32 = np.concatenate(
            [vat, qb[:, :, sl].reshape(P, HC * BPC)], axis=1
        ).astype(np.float32)
        in_maps.append(
            {
                "keysT16": np.ascontiguousarray(keysT16[:, sl]),
                "consts16": np.ascontiguousarray(uaT.reshape(P, HC * H)),
                "consts32": np.ascontiguousarray(consts32),
            }
        )
    return in_maps


def run(inputs, trace=False, **kw):
    nc = _get_nc()
    in_maps = _host_prep(inputs)
    res = run_bass_kernel_spmd(
        nc, in_maps, core_ids=list(range(NCORES)), trace=trace, **kw
    )
    outs = []
    for c in range(NCORES):
        raw = res.results[c]["out_raw"]  # [P, BPC, HC+1]
        parts = raw[:, :, 0:HC]          # [P, BPC, HC]
        den = raw[0, :, HC]              # [BPC]
        o = parts.transpose(1, 2, 0).reshape(BPC, H) / den[:, None]
        outs.append(o)
    out = np.concatenate(outs, axis=0)[:, None, :]  # [B, 1, H]
    return out, res


def kernel(**inputs):
    out, _ = run(inputs)
    return out
